# revision 1
# baseline (speedup 1.0000x reference)
"""Bit-serial conv2d (CIM emulation) for Trainium2, data-parallel over 8 NeuronCores.

Reference math per bit-plane i of int8 input x:
    plane_i = (x >> i) & 1  (two's complement bit)
    y_i = conv2d(plane_i, W, VALID)          # N,64,112,112 -> N,128,110,110
    q_i = 8 * round(y_i / 8)                 # clip inactive for this data
    out = sum_i s_i * q_i + bias,  s_i = 2^i (i<7), -128 (i=7)

Per core (2 of the 16 images):
  - x ships as uint8; bit-planes extracted on-device (DVE shift+and).
  - conv as flat matmuls over the flattened 112x112 image; tap (kh,kw) is a
    shifted read at offset kh*112+kw; junk columns w=110,111 are dropped at
    the output DMA.
  - float32r matmuls (full PE rate; stationary operand kept at 12 explicit
    mantissa bits, RTN). High bit-planes use a 2-term split: hi = weights
    truncated to 10 mantissa bits (exactly representable), lo = residual;
    combined error ~2^-23 relative. Low planes tolerate 1-term (~1.2e-4).
  - K packing: image rows duplicated into partitions 0-63 (x) and 64-127
    (x+112), fusing taps kh=0,1 into K=128 matmuls; kh=2 runs K=64. Odd
    planes use the swapped layout so their kh=2 matmuls sit on PE row-groups
    0-1 while even planes' sit on 2-3 -- the PE overlaps them.
  - bit-plane u8 -> f32r conversion rides on casting SWDGE DMAs (gpsimd).
  - quantize: ACT computes s_i*(y/8) + s_i*M (M = 1.5*2^23, magic rounding
    at granularity s_i since s_i is a power of two); one fused DVE op then
    does acc = (t - s_i*M) + acc. Bias is folded into plane 0's constant.
"""
import sys
sys.path.insert(0, '/opt/trn_rl_repo')
import numpy as np
import concourse.bass as bass
import concourse.mybir as mybir
from concourse import tile
from concourse.bass_utils import run_bass_kernel_spmd
from concourse.alu_op_type import AluOpType

MMAGIC = float(1.5 * 2 ** 23)
W = 112
FL = W * W              # 12544
L = FL + 4              # padded flat length (max read = p+226 <= 12545)
HOUT = 110
NFLAT = HOUT * W        # 12320 flat outputs, w=110,111 junk
GN = 512
GROUPS = [(q, min(GN, NFLAT - q)) for q in range(0, NFLAT, GN)]
NCORES = 8
IMGS = 2
CAST_PAD = 232
# matmul terms per bit-plane: 1 = single full-precision f32r (12-bit RTN),
# 2 = hi(10-bit exact) + lo residual
TERMS = (1, 1, 1, 1, 1, 2, 2, 2)
SCALES = tuple(float(-1024.0 if i == 7 else 8.0 * 2 ** i) for i in range(8))


def _split_sync_waits(nc, max_waits=1):
    """walrus rejects >1 semaphore wait per instruction; hoist excess waits
    onto same-engine NoOps inserted just before."""
    eng = {mybir.EngineType.PE, mybir.EngineType.Activation, mybir.EngineType.DVE,
           mybir.EngineType.Pool, mybir.EngineType.SP}
    k = [0]
    for f in nc.m.functions:
        for blk in f.blocks:
            out, changed = [], False
            for inst in blk.instructions:
                si = inst.sync_info
                waits = list(si.on_wait) if (si and si.on_wait) else []
                if len(waits) > max_waits and inst.engine in eng:
                    excess, keep = waits[:-max_waits], waits[-max_waits:]
                    for i in range(0, len(excess), max_waits):
                        nop = mybir.InstNoOp(name=f"waitsplit_{k[0]}", ins=[], outs=[])
                        k[0] += 1
                        nop.engine = inst.engine
                        nop.sync_info = mybir.SyncInfo(
                            on_wait=excess[i:i + max_waits], on_update=[])
                        out.append(nop)
                    si.on_wait = keep
                    inst.sync_info = si
                    changed = True
                out.append(inst)
            if changed:
                blk.instructions = out
    return k[0]


def _trunc10(w):
    u = np.ascontiguousarray(w, np.float32).view(np.uint32)
    return (u & np.uint32(0xFFFFE000)).view(np.float32).reshape(w.shape)


def _pack_weights(w8):
    """w8: [128,64,3,3] f32 (pre-divided by 8). lhsT packs per term set."""
    hi = _trunc10(w8)
    sets = {"full": w8, "hi": hi, "lo": (w8 - hi).astype(np.float32)}
    out = {}
    for term, wt in sets.items():
        pe = np.zeros((128, 384), np.float32)
        po = np.zeros((128, 384), np.float32)
        se = np.zeros((64, 384), np.float32)
        for kw in range(3):
            pe[:64, kw * 128:(kw + 1) * 128] = wt[:, :, 0, kw].T
            pe[64:, kw * 128:(kw + 1) * 128] = wt[:, :, 1, kw].T
            po[:64, kw * 128:(kw + 1) * 128] = wt[:, :, 1, kw].T
            po[64:, kw * 128:(kw + 1) * 128] = wt[:, :, 0, kw].T
            se[:, kw * 128:(kw + 1) * 128] = wt[:, :, 2, kw].T
        out[f"pair_e_{term}"] = pe
        out[f"pair_o_{term}"] = po
        out[f"solo_{term}"] = se
    return out


_BUILT = {}


def _build():
    nc = bass.Bass("TRN2", target_bir_lowering=False, debug=False,
                   num_devices=NCORES)
    f32r = mybir.dt.float32r
    u8 = mybir.dt.uint8
    f32 = mybir.dt.float32

    xu_d = nc.dram_tensor("xu", [IMGS, 64, FL], u8, kind="ExternalInput").ap()
    wd = {}
    for term in ("full", "hi", "lo"):
        for pre in ("pair_e", "pair_o"):
            nm = f"{pre}_{term}"
            wd[nm] = nc.dram_tensor(nm, [128, 384], f32r, kind="ExternalInput").ap()
        nm = f"solo_{term}"
        wd[nm] = nc.dram_tensor(nm, [64, 384], f32r, kind="ExternalInput").ap()
    c0_d = nc.dram_tensor("c0", [128, 1], f32, kind="ExternalInput").ap()
    out_d = nc.dram_tensor("out", [IMGS, 128, HOUT, HOUT], f32,
                           kind="ExternalOutput").ap()

    with tile.TileContext(nc) as tc:
        with tc.tile_pool(name="const", bufs=1) as pc_, \
             tc.tile_pool(name="img", bufs=2) as pimg, \
             tc.tile_pool(name="accp", bufs=1) as pacc, \
             tc.tile_pool(name="pb", bufs=2) as ppb, \
             tc.tile_pool(name="cs", bufs=3) as pcs, \
             tc.tile_pool(name="qq", bufs=3) as pq, \
             tc.tile_pool(name="psum", bufs=3, space="PSUM") as pps:

            wt = {}
            for term in ("full", "hi", "lo"):
                for pre in ("pair_e", "pair_o"):
                    nm = f"{pre}_{term}"
                    t = pc_.tile([128, 384], f32r, tag=nm)
                    nc.sync.dma_start(t[:], wd[nm][:])
                    wt[nm] = t
                nm = f"solo_{term}"
                t = pc_.tile([128, 384], f32r, tag=nm)
                nc.sync.dma_start(t[64:128, :], wd[nm][:])
                nc.sync.dma_start(t[0:64, :], wd[nm][:])
                wt[nm] = t
            c0_t = pc_.tile([128, 1], f32, tag="c0")
            nc.sync.dma_start(c0_t[:], c0_d[:])

            for img in range(IMGS):
                XU = pimg.tile([128, L], u8, tag="xu")
                XUs = pimg.tile([128, L], u8, tag="xus")
                nc.sync.dma_start(XU[0:64, 0:FL], xu_d[img])
                nc.sync.dma_start(XU[64:128, 0:FL - W], xu_d[img, :, W:])
                nc.sync.dma_start(XUs[0:64, 0:FL - W], xu_d[img, :, W:])
                nc.sync.dma_start(XUs[64:128, 0:FL], xu_d[img])
                nc.vector.memset(XU[0:64, FL:L], 0)
                nc.vector.memset(XU[64:128, FL - W:L], 0)
                nc.vector.memset(XUs[0:64, FL - W:L], 0)
                nc.vector.memset(XUs[64:128, FL:L], 0)

                acc = pacc.tile([128, NFLAT], f32, tag="acc")

                for pi in range(4):
                    ie, io = 2 * pi, 2 * pi + 1
                    PBe = ppb.tile([128, L], u8, tag="pbe")
                    nc.vector.tensor_scalar(PBe[:], XU[:], ie, 1,
                                            AluOpType.logical_shift_right,
                                            AluOpType.bitwise_and)
                    PBo = ppb.tile([128, L], u8, tag="pbo")
                    nc.vector.tensor_scalar(PBo[:], XUs[:], io, 1,
                                            AluOpType.logical_shift_right,
                                            AluOpType.bitwise_and)
                    te = ("full",) if TERMS[ie] == 1 else ("hi", "lo")
                    to = ("full",) if TERMS[io] == 1 else ("hi", "lo")

                    for (q0, gn) in GROUPS:
                        wcast = min(gn + CAST_PAD, L - q0)
                        CSe = pcs.tile([128, GN + CAST_PAD], f32r, tag="cse")
                        nc.gpsimd.dma_start(CSe[:, 0:wcast], PBe[:, q0:q0 + wcast])
                        CSo = pcs.tile([128, GN + CAST_PAD], f32r, tag="cso")
                        nc.gpsimd.dma_start(CSo[:, 0:wcast], PBo[:, q0:q0 + wcast])

                        ye = pps.tile([128, GN], f32, tag="ype")
                        yo = pps.tile([128, GN], f32, tag="ypo")
                        # kh0+kh1 fused K=128 matmuls
                        for k, term in enumerate(te):
                            for kw in range(3):
                                nc.tensor.matmul(
                                    ye[:, 0:gn],
                                    wt[f"pair_e_{term}"][:, kw * 128:(kw + 1) * 128],
                                    CSe[:, kw:kw + gn],
                                    start=(k == 0 and kw == 0), stop=False)
                        for k, term in enumerate(to):
                            for kw in range(3):
                                nc.tensor.matmul(
                                    yo[:, 0:gn],
                                    wt[f"pair_o_{term}"][:, kw * 128:(kw + 1) * 128],
                                    CSo[:, kw:kw + gn],
                                    start=(k == 0 and kw == 0), stop=False)
                        # kh2 K=64: even rows 64-127, odd rows 0-63 (concurrent)
                        ne, no = len(te), len(to)
                        for j in range(max(ne, no) * 3):
                            k, kw = divmod(j, 3)
                            if k < ne:
                                term = te[k]
                                nc.tensor.matmul(
                                    ye[:, 0:gn],
                                    wt[f"solo_{term}"][64:128, kw * 128:(kw + 1) * 128],
                                    CSe[64:128, W + kw:W + kw + gn],
                                    start=False, stop=(j == ne * 3 - 1))
                            if k < no:
                                term = to[k]
                                nc.tensor.matmul(
                                    yo[:, 0:gn],
                                    wt[f"solo_{term}"][0:64, kw * 128:(kw + 1) * 128],
                                    CSo[0:64, W + kw:W + kw + gn],
                                    start=False, stop=(j == no * 3 - 1))
                        for plane, yp in ((ie, ye), (io, yo)):
                            s_i = SCALES[plane]
                            tq = pq.tile([128, GN], f32, tag="tq")
                            nc.scalar.activation(tq[:, 0:gn], yp[:, 0:gn],
                                                 mybir.ActivationFunctionType.Copy,
                                                 bias=MMAGIC * s_i, scale=s_i)
                            aslice = acc[:, q0:q0 + gn]
                            if plane == 0:
                                # acc = t - (M*s0 - bias)
                                nc.vector.tensor_scalar(aslice, tq[:, 0:gn],
                                                        c0_t[:], None,
                                                        AluOpType.subtract)
                            else:
                                nc.vector.scalar_tensor_tensor(
                                    aslice, tq[:, 0:gn], MMAGIC * s_i, aslice,
                                    AluOpType.subtract, AluOpType.add)

                av = acc[:].rearrange("p (h w) -> p h w", w=W)[:, 0:HOUT, 0:HOUT]
                nc.sync.dma_start(out_d[img], av)

    _split_sync_waits(nc)
    return nc


def _prep(x, weight, bias):
    xi = np.clip(x, -128, 127).astype(np.int8).view(np.uint8)
    xu = np.ascontiguousarray(xi.reshape(16, 64, FL))
    w8 = (np.asarray(weight, np.float32) / np.float32(8.0)).astype(np.float32)
    wp = _pack_weights(w8)
    c0 = (np.float32(MMAGIC * SCALES[0])
          - np.asarray(bias, np.float32)).reshape(128, 1)
    shared = {**{k: np.ascontiguousarray(v) for k, v in wp.items()},
              "c0": np.ascontiguousarray(c0.astype(np.float32))}
    in_maps = []
    for c in range(NCORES):
        m = dict(shared)
        m["xu"] = np.ascontiguousarray(xu[c * IMGS:(c + 1) * IMGS])
        in_maps.append(m)
    return in_maps


def get_nc():
    if "nc" not in _BUILT:
        _BUILT["nc"] = _build()
    return _BUILT["nc"]


def kernel(x, weight, bias, _trace=False, _tmpdir=None):
    nc = get_nc()
    in_maps = _prep(x, weight, bias)
    br = run_bass_kernel_spmd(nc, in_maps, list(range(NCORES)),
                              trace=_trace, tmpdir=_tmpdir)
    out = np.concatenate([r["out"] for r in br.results], axis=0)
    if _trace:
        kernel.last_results = br
    return out.astype(np.float32)



# revision 2
# speedup vs baseline: 1.0024x; 1.0024x over previous
"""Bit-serial conv2d (CIM emulation) for Trainium2, 8 NeuronCores data-parallel.

Reference math per bit-plane i of int8 input x:
    plane_i = (x >> i) & 1
    y_i = conv2d(plane_i, W, VALID)          # N,64,112,112 -> N,128,110,110
    out = sum_i s_i * 8*round(y_i/8) + bias,  s_i = 2^i (i<7), -128 (i=7)

Per core (2 of the 16 images). Design:
  - Bit-planes extracted on HOST, shipped as fp16 (0/1 exact, 2B/value).
    fp16 weights too: stationary RTN error 2^-11, measured rel err 1.2e-2
    (gate 2e-2); matmuls run at full PE rate.
  - Single plane layout per chunk [128, w]: rows 0-63 = plane@c, rows
    64-127 = plane@c+112. Pair taps (kh=0,1) read rows 0-127 at +kw (K=128);
    solo taps (kh=2) read rows 0-63 at +224+kw (even planes) or rows 64-127
    at +112+kw (odd planes) as K=64 matmuls on disjoint PE row groups --
    they co-schedule 2-wide (measured 98ns/MM vs 183 serial).
  - Work tiled as 7 chunks/image x 4 groups x 440 cols; each PSUM bank is
    its OWN tile object so the tile framework tracks WAR per bank (a shared
    tile serializes every group's drain against the next group's matmuls).
  - Stationary-major pair matmul order + a post-pass that drops consecutive
    InstLdweights reloading the identical AP (walrus --enable-ldw-opt
    rejects explicit fp16 ldweights, so dedupe here instead).
  - Drains split: even plane quantize on ACT, odd on DVE; magic-number
    rounding (add 1.5*2^23*s_i then subtract) implements 8*round(y/8)*s_i.
    Accumulates (DVE) are deferred one pair so PSUM-freeing drains stay at
    the front of the DVE FIFO. Per-chunk contiguous out-DMA (width 12320 =
    110x112 incl 2 junk cols/row, stripped on host).
"""
import sys
sys.path.insert(0, '/opt/trn_rl_repo')
import os
import numpy as np
import ml_dtypes
import concourse.bass as bass
import concourse.mybir as mybir
from concourse import tile
from concourse.bass_utils import run_bass_kernel_spmd
from concourse.alu_op_type import AluOpType

MMAGIC = float(1.5 * 2 ** 23)
W = 112
FL = W * W                  # 12544
LW = FL + 4                 # padded host plane width
HOUT = 110
NFLAT = HOUT * W            # 12320
GN = 440                    # cols per PSUM group (<=512)
NG = 4                      # groups per chunk
CHUNK = GN * NG             # 1760; 7 chunks per image exactly
NCHUNK = NFLAT // CHUNK     # 7
XF_W = CHUNK + 226 + 2      # 1988 (rows 0-63 read up to +226+439)
XF_WH = CHUNK + 114 + 2     # 1876 (rows 64-127 read up to +112+2+439)
NCORES = 8
IMGS = 2
SCALES = tuple(float(-1024.0 if i == 7 else 8.0 * 2 ** i) for i in range(8))
# planes using a 2-term (hi10-exact + lo) split of the stationary weights
TWO_TERM = ()

_LDW_PATCHED = [False]


def _patch_ldw_opt():
    """Flip walrus' --enable-ldw-opt to true (redundant LDWEIGHTS elision)."""
    if _LDW_PATCHED[0] or os.environ.get("NO_LDW_OPT"):
        return
    import concourse.bass_utils as bu
    orig = bu.run_command

    def patched(argv, **kw):
        argv = ["--enable-ldw-opt=true" if a == "--enable-ldw-opt=false" else a
                for a in argv]
        return orig(argv, **kw)

    bu.run_command = patched
    _LDW_PATCHED[0] = True


def _split_sync_waits(nc, max_waits=1):
    """walrus rejects >1 semaphore wait per instruction; hoist excess waits
    onto same-engine NoOps inserted just before."""
    eng = {mybir.EngineType.PE, mybir.EngineType.Activation, mybir.EngineType.DVE,
           mybir.EngineType.Pool, mybir.EngineType.SP}
    k = [0]
    for f in nc.m.functions:
        for blk in f.blocks:
            out, changed = [], False
            for inst in blk.instructions:
                si = inst.sync_info
                waits = list(si.on_wait) if (si and si.on_wait) else []
                if len(waits) > max_waits and inst.engine in eng:
                    excess, keep = waits[:-max_waits], waits[-max_waits:]
                    for i in range(0, len(excess), max_waits):
                        nop = mybir.InstNoOp(name=f"waitsplit_{k[0]}", ins=[], outs=[])
                        k[0] += 1
                        nop.engine = inst.engine
                        nop.sync_info = mybir.SyncInfo(
                            on_wait=excess[i:i + max_waits], on_update=[])
                        out.append(nop)
                    si.on_wait = keep
                    inst.sync_info = si
                    changed = True
                out.append(inst)
            if changed:
                blk.instructions = out
    return k[0]


def _ap_key(pap):
    ap = pap.bass_ap
    return (ap.tensor.name, ap.offset, str(ap.ap), str(pap.dtype))


def _dedupe_ldweights(nc):
    """Replace consecutive InstLdweights that reload the identical weights AP
    with PE NoOps (sync_info preserved). Matmuls between them keep the loaded
    stationary, so the reload is pure overhead."""
    n = 0
    for f in nc.m.functions:
        for blk in f.blocks:
            cur = None
            out = []
            for inst in blk.instructions:
                if isinstance(inst, mybir.InstLdweights):
                    key = _ap_key(inst.ins[0])
                    if key == cur:
                        n += 1
                        si = inst.sync_info
                        if si is not None and (si.on_wait or si.on_update):
                            nop = mybir.InstNoOp(name=f"ldwdup_{n}",
                                                 ins=[], outs=[])
                            nop.engine = inst.engine
                            nop.sync_info = si
                            out.append(nop)
                        continue
                    cur = key
                elif isinstance(inst, (mybir.InstDrain,)):
                    cur = None  # conservatively assume the array state resets
                out.append(inst)
            blk.instructions = out
    return n


def _trunc10(w):
    u = np.ascontiguousarray(w, np.float32).view(np.uint32)
    return (u & np.uint32(0xFFFFE000)).view(np.float32).reshape(w.shape)


def _pack_weights(w8):
    """w8: [128,64,3,3] f32 (pre-divided by 8) -> stationary tiles."""
    sets = {"full": w8}
    if TWO_TERM:
        hi = w8.astype(np.float16).astype(np.float32)
        sets["hi"] = hi
        sets["lo"] = (w8 - hi).astype(np.float32)
    out = {}
    for term, wt in sets.items():
        pair = np.zeros((128, 384), np.float32)
        solo = np.zeros((128, 384), np.float32)
        for kw in range(3):
            pair[:64, kw * 128:(kw + 1) * 128] = wt[:, :, 0, kw].T
            pair[64:, kw * 128:(kw + 1) * 128] = wt[:, :, 1, kw].T
            solo[:64, kw * 128:(kw + 1) * 128] = wt[:, :, 2, kw].T
            solo[64:, kw * 128:(kw + 1) * 128] = wt[:, :, 2, kw].T
        out[f"pair_{term}"] = pair
        out[f"solo_{term}"] = solo
    return out


_BUILT = {}


def _build():
    # NOTE: walrus --enable-ldw-opt chokes on explicit InstLdweights (fp16
    # path); we dedupe redundant loads ourselves instead.
    nc = bass.Bass("TRN2", target_bir_lowering=False, debug=False,
                   num_devices=NCORES)
    f16 = mybir.dt.float16
    f32 = mybir.dt.float32

    pl_d = nc.dram_tensor("pl", [IMGS, 8, 64, LW], f16,
                          kind="ExternalInput").ap()
    wd = {}
    for term in (("full",) if not TWO_TERM else ("full", "hi", "lo")):
        for pre in ("pair", "solo"):
            nm = f"{pre}_{term}"
            wd[nm] = nc.dram_tensor(nm, [128, 384], f16,
                                    kind="ExternalInput").ap()
    c0_d = nc.dram_tensor("c0", [128, 1], f32, kind="ExternalInput").ap()
    out_d = nc.dram_tensor("out", [IMGS, 128, NFLAT], f32,
                           kind="ExternalOutput").ap()

    with tile.TileContext(nc) as tc:
        with tc.tile_pool(name="const", bufs=1) as pc_, \
             tc.tile_pool(name="xf", bufs=3) as pxf, \
             tc.tile_pool(name="accp", bufs=2) as pacc, \
             tc.tile_pool(name="qq", bufs=4) as pq, \
             tc.tile_pool(name="psum", bufs=1, space="PSUM") as pps:

            wt = {}
            for nm, d in wd.items():
                t = pc_.tile([128, 384], f16, tag=nm, name=f"w_{nm}")
                nc.sync.dma_start(t[:], d[:])
                wt[nm] = t
            c0_t = pc_.tile([128, 1], f32, tag="c0")
            nc.sync.dma_start(c0_t[:], c0_d[:])

            def emit_accums(p):
                acc_, tqe_, tqo_, ie_, io_ = p
                s_e_, s_o_ = SCALES[ie_], SCALES[io_]
                if ie_ == 0:
                    nc.vector.tensor_scalar(
                        acc_[:], tqe_[:], c0_t[:], None,
                        AluOpType.subtract)
                else:
                    nc.vector.scalar_tensor_tensor(
                        acc_[:], tqe_[:], MMAGIC * s_e_, acc_[:],
                        AluOpType.subtract, AluOpType.add)
                nc.vector.scalar_tensor_tensor(
                    acc_[:], tqo_[:], MMAGIC * s_o_, acc_[:],
                    AluOpType.subtract, AluOpType.add)

            pending = None      # accums deferred past next pair's drains
            pending_out = None  # (img, c0, acc) out-DMA after pair-3 accums
            for img in range(IMGS):
                for ci in range(NCHUNK):
                    c0 = ci * CHUNK
                    acc = pacc.tile([128, CHUNK], f32, tag="acc")
                    for pi in range(4):
                        ie, io = 2 * pi, 2 * pi + 1
                        XFe = pxf.tile([128, XF_W], f16, tag="xfe")
                        XFo = pxf.tile([128, XF_W], f16, tag="xfo")
                        nc.sync.dma_start(XFe[0:64, 0:XF_W],
                                          pl_d[img, ie, :, c0:c0 + XF_W])
                        nc.sync.dma_start(XFe[64:128, 0:XF_WH],
                                          pl_d[img, ie, :, c0 + W:c0 + W + XF_WH])
                        nc.sync.dma_start(XFo[0:64, 0:XF_W],
                                          pl_d[img, io, :, c0:c0 + XF_W])
                        nc.sync.dma_start(XFo[64:128, 0:XF_WH],
                                          pl_d[img, io, :, c0 + W:c0 + W + XF_WH])

                        peb = [pps.tile([128, 512], f32, tag=f"pe{g}",
                                        name=f"pe{g}") for g in range(NG)]
                        pob = [pps.tile([128, 512], f32, tag=f"po{g}",
                                        name=f"po{g}") for g in range(NG)]

                        te = ("full",) if ie not in TWO_TERM else ("hi", "lo")
                        to = ("full",) if io not in TWO_TERM else ("hi", "lo")

                        # pair taps kh=0,1: stationary-major, 4+4 matmuls
                        # per LDW (ldw-opt elides reloads)
                        first = True
                        for kw in range(3):
                            for term in te:
                                s = wt[f"pair_{term}"][:, kw * 128:(kw + 1) * 128]
                                for g in range(NG):
                                    nc.tensor.matmul(
                                        peb[g][:, 0:GN], s,
                                        XFe[:, g * GN + kw:g * GN + kw + GN],
                                        start=first and term == te[0],
                                        stop=False)
                            for term in to:
                                s = wt[f"pair_{term}"][:, kw * 128:(kw + 1) * 128]
                                for g in range(NG):
                                    nc.tensor.matmul(
                                        pob[g][:, 0:GN], s,
                                        XFo[:, g * GN + kw:g * GN + kw + GN],
                                        start=first and term == to[0],
                                        stop=False)
                            first = False

                        # solo taps kh=2: even plane on rows 0-63 (@+224+kw),
                        # odd plane on rows 64-127 (@+112+kw); e/o interleaved
                        # so the PE co-schedules the row halves. Group-major
                        # so bank stops stagger and drains pipeline early.
                        ne, no = len(te), len(to)
                        tqe = pq.tile([128, CHUNK], f32, tag="tqe")
                        tqo = pq.tile([128, CHUNK], f32, tag="tqo")
                        s_e, s_o = SCALES[ie], SCALES[io]
                        for g in range(NG):
                            for kw in range(3):
                                for ti in range(max(ne, no)):
                                    if ti < ne:
                                        s = wt[f"solo_{te[ti]}"][0:64,
                                                                 kw * 128:(kw + 1) * 128]
                                        nc.tensor.matmul(
                                            peb[g][:, 0:GN], s,
                                            XFe[0:64,
                                                g * GN + 224 + kw:g * GN + 224 + kw + GN],
                                            start=False,
                                            stop=(kw == 2 and ti == ne - 1))
                                    if ti < no:
                                        s = wt[f"solo_{to[ti]}"][64:128,
                                                                 kw * 128:(kw + 1) * 128]
                                        nc.tensor.matmul(
                                            pob[g][:, 0:GN], s,
                                            XFo[64:128,
                                                g * GN + W + kw:g * GN + W + kw + GN],
                                            start=False,
                                            stop=(kw == 2 and ti == no - 1))
                            # drain bank g immediately: even plane on ACT,
                            # odd plane on DVE (PSUM frees without a single-
                            # engine drain queue at the chunk boundary)
                            gsl = slice(g * GN, (g + 1) * GN)
                            nc.scalar.activation(
                                tqe[:, gsl], peb[g][:, 0:GN],
                                mybir.ActivationFunctionType.Copy,
                                bias=MMAGIC * s_e, scale=s_e)
                            nc.vector.tensor_scalar(
                                tqo[:, gsl], pob[g][:, 0:GN],
                                s_o, MMAGIC * s_o,
                                AluOpType.mult, AluOpType.add)

                        # deferred accumulate: previous pair's ops go
                        # behind this pair's PSUM-freeing drains in the
                        # DVE FIFO
                        if pending is not None:
                            emit_accums(pending)
                            if pending_out is not None:
                                oimg, oc0, oacc = pending_out
                                nc.sync.dma_start(
                                    out_d[oimg, :, oc0:oc0 + CHUNK], oacc[:])
                                pending_out = None
                        pending = (acc, tqe, tqo, ie, io)
                        if pi == 3:
                            pending_out = (img, c0, acc)

            emit_accums(pending)
            oimg, oc0, oacc = pending_out
            nc.sync.dma_start(out_d[oimg, :, oc0:oc0 + CHUNK], oacc[:])

    nd = _dedupe_ldweights(nc)
    _split_sync_waits(nc)
    if os.environ.get("V3_DEBUG"):
        print(f"[v3] deduped {nd} ldweights")
    return nc


def _prep(x, weight, bias):
    xi = np.clip(x, -128, 127).astype(np.int8)
    # bit-planes as bf16 bit patterns (1.0 = 0x3F80), flat per image
    xf = xi.reshape(16, 64, FL)
    planes = np.zeros((16, 8, 64, LW), np.float16)
    for i in range(8):
        planes[:, i, :, 0:FL] = ((xf >> i) & 1).astype(np.float16)

    w8 = (np.asarray(weight, np.float32) / np.float32(8.0)).astype(np.float32)
    wp = {k: v.astype(np.float16) for k, v in _pack_weights(w8).items()}
    c0 = (np.float32(MMAGIC * SCALES[0])
          - np.asarray(bias, np.float32)).reshape(128, 1)
    shared = {**{k: np.ascontiguousarray(v) for k, v in wp.items()},
              "c0": np.ascontiguousarray(c0.astype(np.float32))}
    in_maps = []
    for c in range(NCORES):
        m = dict(shared)
        m["pl"] = planes[c * IMGS:(c + 1) * IMGS]
        in_maps.append(m)
    return in_maps


def get_nc():
    if "nc" not in _BUILT:
        _BUILT["nc"] = _build()
    return _BUILT["nc"]


def kernel(x, weight, bias, _trace=False, _tmpdir=None):
    nc = get_nc()
    in_maps = _prep(x, weight, bias)
    br = run_bass_kernel_spmd(nc, in_maps, list(range(NCORES)),
                              trace=_trace, tmpdir=_tmpdir)
    out = np.concatenate([r["out"] for r in br.results], axis=0)
    out = out.reshape(-1, 128, HOUT, W)[:, :, :, 0:HOUT]
    if _trace:
        kernel.last_results = br
    return out.astype(np.float32)


# revision 3
# speedup vs baseline: 1.0134x; 1.0109x over previous
"""Bit-serial conv2d (CIM emulation) for Trainium2, 8 NeuronCores data-parallel.

Reference math per bit-plane i of int8 input x:
    plane_i = (x >> i) & 1
    y_i = conv2d(plane_i, W, VALID)          # N,64,112,112 -> N,128,110,110
    out = sum_i s_i * 8*round(y_i/8) + bias,  s_i = 2^i (i<7), -128 (i=7)

Per core (2 of the 16 images). Design:
  - Bit-planes extracted on HOST, shipped as fp16 (0/1 exact, 2B/value).
    fp16 weights too: stationary RTN error 2^-11, measured rel err 1.2e-2
    (gate 2e-2); matmuls run at full PE rate.
  - Single plane layout per chunk [128, w]: rows 0-63 = plane@c, rows
    64-127 = plane@c+112. Pair taps (kh=0,1) read rows 0-127 at +kw (K=128);
    solo taps (kh=2) read rows 0-63 at +224+kw (even planes) or rows 64-127
    at +112+kw (odd planes) as K=64 matmuls on disjoint PE row groups --
    they co-schedule 2-wide (measured 98ns/MM vs 183 serial).
  - Work tiled as 7 chunks/image x 4 groups x 440 cols; each PSUM bank is
    its OWN tile object so the tile framework tracks WAR per bank (a shared
    tile serializes every group's drain against the next group's matmuls).
  - Stationary-major pair matmul order + a post-pass that drops consecutive
    InstLdweights reloading the identical AP (walrus --enable-ldw-opt
    rejects explicit fp16 ldweights, so dedupe here instead).
  - Drains split: even plane quantize on ACT, odd on DVE; magic-number
    rounding (add 1.5*2^23*s_i then subtract) implements 8*round(y/8)*s_i.
    Accumulates (DVE) are deferred one pair so PSUM-freeing drains stay at
    the front of the DVE FIFO. Per-chunk contiguous out-DMA (width 12320 =
    110x112 incl 2 junk cols/row, stripped on host).
"""
import sys
sys.path.insert(0, '/opt/trn_rl_repo')
import os
import numpy as np
import concourse.bass as bass
import concourse.mybir as mybir
from concourse import tile
from concourse.bass_utils import run_bass_kernel_spmd
from concourse.alu_op_type import AluOpType

MMAGIC = float(1.5 * 2 ** 23)
W = 112
FL = W * W                  # 12544
LW = FL + 4                 # padded host plane width
HOUT = 110
NFLAT = HOUT * W            # 12320
GN = 440                    # cols per PSUM group (<=512)
NG = 4                      # groups per chunk
CHUNK = GN * NG             # 1760; 7 chunks per image exactly
NCHUNK = NFLAT // CHUNK     # 7
XF_W = CHUNK + 226 + 2      # 1988 (rows 0-63 read up to +226+439)
XF_WH = CHUNK + 114 + 2     # 1876 (rows 64-127 read up to +112+2+439)
NCORES = 8
IMGS = 2
SCALES = tuple(float(-1024.0 if i == 7 else 8.0 * 2 ** i) for i in range(8))
# planes using a 2-term (hi10-exact + lo) split of the stationary weights
TWO_TERM = ()

def _split_sync_waits(nc, max_waits=1):
    """walrus rejects >1 semaphore wait per instruction; hoist excess waits
    onto same-engine NoOps inserted just before."""
    eng = {mybir.EngineType.PE, mybir.EngineType.Activation, mybir.EngineType.DVE,
           mybir.EngineType.Pool, mybir.EngineType.SP}
    k = [0]
    for f in nc.m.functions:
        for blk in f.blocks:
            out, changed = [], False
            for inst in blk.instructions:
                si = inst.sync_info
                waits = list(si.on_wait) if (si and si.on_wait) else []
                if len(waits) > max_waits and inst.engine in eng:
                    excess, keep = waits[:-max_waits], waits[-max_waits:]
                    for i in range(0, len(excess), max_waits):
                        nop = mybir.InstNoOp(name=f"waitsplit_{k[0]}", ins=[], outs=[])
                        k[0] += 1
                        nop.engine = inst.engine
                        nop.sync_info = mybir.SyncInfo(
                            on_wait=excess[i:i + max_waits], on_update=[])
                        out.append(nop)
                    si.on_wait = keep
                    inst.sync_info = si
                    changed = True
                out.append(inst)
            if changed:
                blk.instructions = out
    return k[0]


def _ap_key(pap):
    ap = pap.bass_ap
    return (ap.tensor.name, ap.offset, str(ap.ap), str(pap.dtype))


def _dedupe_ldweights(nc):
    """Replace consecutive InstLdweights that reload the identical weights AP
    with PE NoOps (sync_info preserved). Matmuls between them keep the loaded
    stationary, so the reload is pure overhead."""
    n = 0
    for f in nc.m.functions:
        for blk in f.blocks:
            cur = None
            out = []
            for inst in blk.instructions:
                if isinstance(inst, mybir.InstLdweights):
                    key = _ap_key(inst.ins[0])
                    if key == cur:
                        n += 1
                        si = inst.sync_info
                        if si is not None and (si.on_wait or si.on_update):
                            nop = mybir.InstNoOp(name=f"ldwdup_{n}",
                                                 ins=[], outs=[])
                            nop.engine = inst.engine
                            nop.sync_info = si
                            out.append(nop)
                        continue
                    cur = key
                elif isinstance(inst, (mybir.InstDrain,)):
                    cur = None  # conservatively assume the array state resets
                out.append(inst)
            blk.instructions = out
    return n


def _trunc10(w):
    u = np.ascontiguousarray(w, np.float32).view(np.uint32)
    return (u & np.uint32(0xFFFFE000)).view(np.float32).reshape(w.shape)


def _pack_weights(w8):
    """w8: [128,64,3,3] f32 (pre-divided by 8) -> stationary tiles."""
    sets = {"full": w8}
    if TWO_TERM:
        hi = w8.astype(np.float16).astype(np.float32)
        sets["hi"] = hi
        sets["lo"] = (w8 - hi).astype(np.float32)
    out = {}
    for term, wt in sets.items():
        pair = np.zeros((128, 384), np.float32)
        solo = np.zeros((128, 384), np.float32)
        for kw in range(3):
            pair[:64, kw * 128:(kw + 1) * 128] = wt[:, :, 0, kw].T
            pair[64:, kw * 128:(kw + 1) * 128] = wt[:, :, 1, kw].T
            solo[:64, kw * 128:(kw + 1) * 128] = wt[:, :, 2, kw].T
            solo[64:, kw * 128:(kw + 1) * 128] = wt[:, :, 2, kw].T
        out[f"pair_{term}"] = pair
        out[f"solo_{term}"] = solo
    return out


_BUILT = {}


def _build():
    # NOTE: walrus --enable-ldw-opt chokes on explicit InstLdweights (fp16
    # path); we dedupe redundant loads ourselves instead.
    nc = bass.Bass("TRN2", target_bir_lowering=False, debug=False,
                   num_devices=NCORES)
    f16 = mybir.dt.float16
    f32 = mybir.dt.float32

    pl_d = nc.dram_tensor("pl", [IMGS, 8, 64, LW], f16,
                          kind="ExternalInput").ap()
    wd = {}
    for term in (("full",) if not TWO_TERM else ("full", "hi", "lo")):
        for pre in ("pair", "solo"):
            nm = f"{pre}_{term}"
            wd[nm] = nc.dram_tensor(nm, [128, 384], f16,
                                    kind="ExternalInput").ap()
    c0_d = nc.dram_tensor("c0", [128, 1], f32, kind="ExternalInput").ap()
    out_d = nc.dram_tensor("out", [IMGS, 128, NFLAT], f32,
                           kind="ExternalOutput").ap()

    with tile.TileContext(nc) as tc:
        with tc.tile_pool(name="const", bufs=1) as pc_, \
             tc.tile_pool(name="xf", bufs=3) as pxf, \
             tc.tile_pool(name="accp", bufs=2) as pacc, \
             tc.tile_pool(name="qq", bufs=4) as pq, \
             tc.tile_pool(name="psum", bufs=1, space="PSUM") as pps:

            wt = {}
            for nm, d in wd.items():
                t = pc_.tile([128, 384], f16, tag=nm, name=f"w_{nm}")
                nc.sync.dma_start(t[:], d[:])
                wt[nm] = t
            c0_t = pc_.tile([128, 1], f32, tag="c0")
            nc.sync.dma_start(c0_t[:], c0_d[:])

            def emit_accums(p):
                acc_, tqe_, tqo_, ie_, io_ = p
                s_e_, s_o_ = SCALES[ie_], SCALES[io_]
                if ie_ == 0:
                    nc.vector.tensor_scalar(
                        acc_[:], tqe_[:], c0_t[:], None,
                        AluOpType.subtract)
                else:
                    nc.vector.scalar_tensor_tensor(
                        acc_[:], tqe_[:], MMAGIC * s_e_, acc_[:],
                        AluOpType.subtract, AluOpType.add)
                nc.vector.scalar_tensor_tensor(
                    acc_[:], tqo_[:], MMAGIC * s_o_, acc_[:],
                    AluOpType.subtract, AluOpType.add)

            pending = None      # accums deferred past next pair's drains
            pending_out = None  # (img, c0, acc) out-DMA after pair-3 accums
            for img in range(IMGS):
                for ci in range(NCHUNK):
                    c0 = ci * CHUNK
                    acc = pacc.tile([128, CHUNK], f32, tag="acc")
                    for pi in range(4):
                        ie, io = 2 * pi, 2 * pi + 1
                        XFe = pxf.tile([128, XF_W], f16, tag="xfe")
                        XFo = pxf.tile([128, XF_W], f16, tag="xfo")
                        nc.sync.dma_start(XFe[0:64, 0:XF_W],
                                          pl_d[img, ie, :, c0:c0 + XF_W])
                        nc.sync.dma_start(XFe[64:128, 0:XF_WH],
                                          pl_d[img, ie, :, c0 + W:c0 + W + XF_WH])
                        nc.sync.dma_start(XFo[0:64, 0:XF_W],
                                          pl_d[img, io, :, c0:c0 + XF_W])
                        nc.sync.dma_start(XFo[64:128, 0:XF_WH],
                                          pl_d[img, io, :, c0 + W:c0 + W + XF_WH])

                        peb = [pps.tile([128, 512], f32, tag=f"pe{g}",
                                        name=f"pe{g}") for g in range(NG)]
                        pob = [pps.tile([128, 512], f32, tag=f"po{g}",
                                        name=f"po{g}") for g in range(NG)]

                        te = ("full",) if ie not in TWO_TERM else ("hi", "lo")
                        to = ("full",) if io not in TWO_TERM else ("hi", "lo")

                        # pair taps kh=0,1: stationary-major, 4+4 matmuls
                        # per LDW (ldw-opt elides reloads)
                        first = True
                        for kw in range(3):
                            for term in te:
                                s = wt[f"pair_{term}"][:, kw * 128:(kw + 1) * 128]
                                for g in range(NG):
                                    nc.tensor.matmul(
                                        peb[g][:, 0:GN], s,
                                        XFe[:, g * GN + kw:g * GN + kw + GN],
                                        start=first and term == te[0],
                                        stop=False)
                            for term in to:
                                s = wt[f"pair_{term}"][:, kw * 128:(kw + 1) * 128]
                                for g in range(NG):
                                    nc.tensor.matmul(
                                        pob[g][:, 0:GN], s,
                                        XFo[:, g * GN + kw:g * GN + kw + GN],
                                        start=first and term == to[0],
                                        stop=False)
                            first = False

                        # solo taps kh=2: even plane on rows 0-63 (@+224+kw),
                        # odd plane on rows 64-127 (@+112+kw); e/o interleaved
                        # so the PE co-schedules the row halves. Group-major
                        # so bank stops stagger and drains pipeline early.
                        ne, no = len(te), len(to)
                        tqe = pq.tile([128, CHUNK], f32, tag="tqe")
                        tqo = pq.tile([128, CHUNK], f32, tag="tqo")
                        s_e, s_o = SCALES[ie], SCALES[io]
                        for g in range(NG):
                            for kw in range(3):
                                for ti in range(max(ne, no)):
                                    if ti < ne:
                                        s = wt[f"solo_{te[ti]}"][0:64,
                                                                 kw * 128:(kw + 1) * 128]
                                        nc.tensor.matmul(
                                            peb[g][:, 0:GN], s,
                                            XFe[0:64,
                                                g * GN + 224 + kw:g * GN + 224 + kw + GN],
                                            start=False,
                                            stop=(kw == 2 and ti == ne - 1))
                                    if ti < no:
                                        s = wt[f"solo_{to[ti]}"][64:128,
                                                                 kw * 128:(kw + 1) * 128]
                                        nc.tensor.matmul(
                                            pob[g][:, 0:GN], s,
                                            XFo[64:128,
                                                g * GN + W + kw:g * GN + W + kw + GN],
                                            start=False,
                                            stop=(kw == 2 and ti == no - 1))
                            # drain bank g immediately: even plane on ACT,
                            # odd plane on DVE (PSUM frees without a single-
                            # engine drain queue at the chunk boundary)
                            gsl = slice(g * GN, (g + 1) * GN)
                            nc.scalar.activation(
                                tqe[:, gsl], peb[g][:, 0:GN],
                                mybir.ActivationFunctionType.Copy,
                                bias=MMAGIC * s_e, scale=s_e)
                            nc.vector.tensor_scalar(
                                tqo[:, gsl], pob[g][:, 0:GN],
                                s_o, MMAGIC * s_o,
                                AluOpType.mult, AluOpType.add)

                        # deferred accumulate: previous pair's ops go
                        # behind this pair's PSUM-freeing drains in the
                        # DVE FIFO
                        if pending is not None:
                            emit_accums(pending)
                            if pending_out is not None:
                                oimg, oc0, oacc = pending_out
                                nc.sync.dma_start(
                                    out_d[oimg, :, oc0:oc0 + CHUNK], oacc[:])
                                pending_out = None
                        pending = (acc, tqe, tqo, ie, io)
                        if pi == 3:
                            pending_out = (img, c0, acc)

            emit_accums(pending)
            oimg, oc0, oacc = pending_out
            nc.sync.dma_start(out_d[oimg, :, oc0:oc0 + CHUNK], oacc[:])

    nd = _dedupe_ldweights(nc)
    _split_sync_waits(nc)
    if os.environ.get("V3_DEBUG"):
        print(f"[v3] deduped {nd} ldweights")
    return nc


def _prep(x, weight, bias):
    xi = np.clip(x, -128, 127).astype(np.int8)
    # bit-planes as bf16 bit patterns (1.0 = 0x3F80), flat per image
    xf = xi.reshape(16, 64, FL)
    planes = np.zeros((16, 8, 64, LW), np.float16)
    for i in range(8):
        planes[:, i, :, 0:FL] = ((xf >> i) & 1).astype(np.float16)

    w8 = (np.asarray(weight, np.float32) / np.float32(8.0)).astype(np.float32)
    wp = {k: v.astype(np.float16) for k, v in _pack_weights(w8).items()}
    c0 = (np.float32(MMAGIC * SCALES[0])
          - np.asarray(bias, np.float32)).reshape(128, 1)
    shared = {**{k: np.ascontiguousarray(v) for k, v in wp.items()},
              "c0": np.ascontiguousarray(c0.astype(np.float32))}
    in_maps = []
    for c in range(NCORES):
        m = dict(shared)
        m["pl"] = planes[c * IMGS:(c + 1) * IMGS]
        in_maps.append(m)
    return in_maps


def get_nc():
    if "nc" not in _BUILT:
        _BUILT["nc"] = _build()
    return _BUILT["nc"]


def kernel(x, weight, bias, _trace=False, _tmpdir=None):
    nc = get_nc()
    in_maps = _prep(x, weight, bias)
    br = run_bass_kernel_spmd(nc, in_maps, list(range(NCORES)),
                              trace=_trace, tmpdir=_tmpdir)
    out = np.concatenate([r["out"] for r in br.results], axis=0)
    out = out.reshape(-1, 128, HOUT, W)[:, :, :, 0:HOUT]
    if _trace:
        kernel.last_results = br
    return out.astype(np.float32)


# revision 4
# speedup vs baseline: 1.0164x; 1.0030x over previous
"""Bit-serial conv2d (CIM emulation) for Trainium2, 8 NeuronCores data-parallel.

Reference math per bit-plane i of int8 input x:
    plane_i = (x >> i) & 1
    y_i = conv2d(plane_i, W, VALID)          # N,64,112,112 -> N,128,110,110
    out = sum_i s_i * 8*round(y_i/8) + bias,  s_i = 2^i (i<7), -128 (i=7)

Per core (2 of the 16 images). Design:
  - Bit-planes extracted on HOST, shipped as fp16 (0/1 exact, 2B/value).
    fp16 weights too: stationary RTN error 2^-11, measured rel err 1.2e-2
    (gate 2e-2); matmuls run at full PE rate.
  - Single plane layout per chunk [128, w]: rows 0-63 = plane@c, rows
    64-127 = plane@c+112. Pair taps (kh=0,1) read rows 0-127 at +kw (K=128);
    solo taps (kh=2) read rows 0-63 at +224+kw (even planes) or rows 64-127
    at +112+kw (odd planes) as K=64 matmuls on disjoint PE row groups --
    they co-schedule 2-wide (measured 98ns/MM vs 183 serial).
  - Work tiled as 7 chunks/image x 4 groups x 440 cols; each PSUM bank is
    its OWN tile object so the tile framework tracks WAR per bank (a shared
    tile serializes every group's drain against the next group's matmuls).
  - Stationary-major pair matmul order + a post-pass that drops consecutive
    InstLdweights reloading the identical AP (walrus --enable-ldw-opt
    rejects explicit fp16 ldweights, so dedupe here instead).
  - Drains split: even plane quantize on ACT, odd on DVE; magic-number
    rounding (add 1.5*2^23*s_i then subtract) implements 8*round(y/8)*s_i.
    Accumulates (DVE) are deferred one pair so PSUM-freeing drains stay at
    the front of the DVE FIFO. Junk image cols w=110,111 are skipped via
    strided moving APs (full rate, probed), so outputs are 110-packed and
    the per-chunk out-DMA is contiguous with no host-side strip.
"""
import sys
sys.path.insert(0, '/opt/trn_rl_repo')
import os
import numpy as np
import concourse.bass as bass
import concourse.mybir as mybir
from concourse import tile
from concourse.bass_utils import run_bass_kernel_spmd
from concourse.alu_op_type import AluOpType

MMAGIC = float(1.5 * 2 ** 23)
W = 112
FL = W * W                  # 12544
LW = FL + 4                 # padded host plane width
HOUT = 110
NFLAT = HOUT * HOUT         # 12100 (junk cols w=110,111 skipped via
                            # strided moving APs; output 110-packed)
NG = 4                      # groups per chunk
CHUNKS = [(0, 16), (16, 16), (32, 16), (48, 16), (64, 16), (80, 16), (96, 14)]
CCMAX = 16 * HOUT           # 1760 packed cols, max per chunk
XF_WMAX = 15 * W + 336      # 2016
NCORES = 8
IMGS = 2
SCALES = tuple(float(-1024.0 if i == 7 else 8.0 * 2 ** i) for i in range(8))
# planes using a 2-term (hi10-exact + lo) split of the stationary weights
TWO_TERM = ()

def _split_sync_waits(nc, max_waits=1):
    """walrus rejects >1 semaphore wait per instruction; hoist excess waits
    onto same-engine NoOps inserted just before."""
    eng = {mybir.EngineType.PE, mybir.EngineType.Activation, mybir.EngineType.DVE,
           mybir.EngineType.Pool, mybir.EngineType.SP}
    k = [0]
    for f in nc.m.functions:
        for blk in f.blocks:
            out, changed = [], False
            for inst in blk.instructions:
                si = inst.sync_info
                waits = list(si.on_wait) if (si and si.on_wait) else []
                if len(waits) > max_waits and inst.engine in eng:
                    excess, keep = waits[:-max_waits], waits[-max_waits:]
                    for i in range(0, len(excess), max_waits):
                        nop = mybir.InstNoOp(name=f"waitsplit_{k[0]}", ins=[], outs=[])
                        k[0] += 1
                        nop.engine = inst.engine
                        nop.sync_info = mybir.SyncInfo(
                            on_wait=excess[i:i + max_waits], on_update=[])
                        out.append(nop)
                    si.on_wait = keep
                    inst.sync_info = si
                    changed = True
                out.append(inst)
            if changed:
                blk.instructions = out
    return k[0]


def _ap_key(pap):
    ap = pap.bass_ap
    return (ap.tensor.name, ap.offset, str(ap.ap), str(pap.dtype))


def _dedupe_ldweights(nc):
    """Replace consecutive InstLdweights that reload the identical weights AP
    with PE NoOps (sync_info preserved). Matmuls between them keep the loaded
    stationary, so the reload is pure overhead."""
    n = 0
    for f in nc.m.functions:
        for blk in f.blocks:
            cur = None
            out = []
            for inst in blk.instructions:
                if isinstance(inst, mybir.InstLdweights):
                    key = _ap_key(inst.ins[0])
                    if key == cur:
                        n += 1
                        si = inst.sync_info
                        if si is not None and (si.on_wait or si.on_update):
                            nop = mybir.InstNoOp(name=f"ldwdup_{n}",
                                                 ins=[], outs=[])
                            nop.engine = inst.engine
                            nop.sync_info = si
                            out.append(nop)
                        continue
                    cur = key
                elif isinstance(inst, (mybir.InstDrain,)):
                    cur = None  # conservatively assume the array state resets
                out.append(inst)
            blk.instructions = out
    return n


def _trunc10(w):
    u = np.ascontiguousarray(w, np.float32).view(np.uint32)
    return (u & np.uint32(0xFFFFE000)).view(np.float32).reshape(w.shape)


def _pack_weights(w8):
    """w8: [128,64,3,3] f32 (pre-divided by 8) -> stationary tiles."""
    sets = {"full": w8}
    if TWO_TERM:
        hi = w8.astype(np.float16).astype(np.float32)
        sets["hi"] = hi
        sets["lo"] = (w8 - hi).astype(np.float32)
    out = {}
    for term, wt in sets.items():
        pair = np.zeros((128, 384), np.float32)
        solo = np.zeros((128, 384), np.float32)
        for kw in range(3):
            pair[:64, kw * 128:(kw + 1) * 128] = wt[:, :, 0, kw].T
            pair[64:, kw * 128:(kw + 1) * 128] = wt[:, :, 1, kw].T
            solo[:64, kw * 128:(kw + 1) * 128] = wt[:, :, 2, kw].T
            solo[64:, kw * 128:(kw + 1) * 128] = wt[:, :, 2, kw].T
        out[f"pair_{term}"] = pair
        out[f"solo_{term}"] = solo
    return out


_BUILT = {}


def _build():
    # NOTE: walrus --enable-ldw-opt chokes on explicit InstLdweights (fp16
    # path); we dedupe redundant loads ourselves instead.
    nc = bass.Bass("TRN2", target_bir_lowering=False, debug=False,
                   num_devices=NCORES)
    f16 = mybir.dt.float16
    f32 = mybir.dt.float32

    pl_d = nc.dram_tensor("pl", [IMGS, 8, 64, LW], f16,
                          kind="ExternalInput").ap()
    wd = {}
    for term in (("full",) if not TWO_TERM else ("full", "hi", "lo")):
        for pre in ("pair", "solo"):
            nm = f"{pre}_{term}"
            wd[nm] = nc.dram_tensor(nm, [128, 384], f16,
                                    kind="ExternalInput").ap()
    c0_d = nc.dram_tensor("c0", [128, 1], f32, kind="ExternalInput").ap()
    out_d = nc.dram_tensor("out", [IMGS, 128, NFLAT], f32,
                           kind="ExternalOutput").ap()

    with tile.TileContext(nc) as tc:
        with tc.tile_pool(name="const", bufs=1) as pc_, \
             tc.tile_pool(name="xf", bufs=3) as pxf, \
             tc.tile_pool(name="accp", bufs=2) as pacc, \
             tc.tile_pool(name="qq", bufs=4) as pq, \
             tc.tile_pool(name="psum", bufs=1, space="PSUM") as pps:

            wt = {}
            for nm, d in wd.items():
                t = pc_.tile([128, 384], f16, tag=nm, name=f"w_{nm}")
                nc.sync.dma_start(t[:], d[:])
                wt[nm] = t
            c0_t = pc_.tile([128, 1], f32, tag="c0")
            nc.sync.dma_start(c0_t[:], c0_d[:])

            def emit_accums(p):
                acc_, tqe_, tqo_, ie_, io_, cc_ = p
                s_e_, s_o_ = SCALES[ie_], SCALES[io_]
                if ie_ == 0:
                    nc.vector.tensor_scalar(
                        acc_[:, 0:cc_], tqe_[:, 0:cc_], c0_t[:], None,
                        AluOpType.subtract)
                else:
                    nc.vector.scalar_tensor_tensor(
                        acc_[:, 0:cc_], tqe_[:, 0:cc_], MMAGIC * s_e_,
                        acc_[:, 0:cc_], AluOpType.subtract, AluOpType.add)
                nc.vector.scalar_tensor_tensor(
                    acc_[:, 0:cc_], tqo_[:, 0:cc_], MMAGIC * s_o_,
                    acc_[:, 0:cc_], AluOpType.subtract, AluOpType.add)

            def mv(t, p0, p1, off, gr, kw):
                return t[p0:p1, off:off + gr * W].rearrange(
                    "p (h w) -> p h w", w=W)[:, :, kw:kw + HOUT]

            pending = None      # accums deferred past next pair's drains
            pending_out = None  # out-DMA emitted after pair-3 accums
            for img in range(IMGS):
                for (row0, nrows) in CHUNKS:
                    ci0 = row0 * W
                    ccols = nrows * HOUT
                    grs = [4, 4, 4, nrows - 12]
                    wa = (nrows - 1) * W + 336
                    wb = (nrows - 1) * W + 224
                    acc = pacc.tile([128, CCMAX], f32, tag="acc")
                    for pi in range(4):
                        ie, io = 2 * pi, 2 * pi + 1
                        XFe = pxf.tile([128, XF_WMAX], f16, tag="xfe")
                        XFo = pxf.tile([128, XF_WMAX], f16, tag="xfo")
                        nc.sync.dma_start(XFe[0:64, 0:wa],
                                          pl_d[img, ie, :, ci0:ci0 + wa])
                        nc.sync.dma_start(XFe[64:128, 0:wb],
                                          pl_d[img, ie, :, ci0 + W:ci0 + W + wb])
                        nc.sync.dma_start(XFo[0:64, 0:wa],
                                          pl_d[img, io, :, ci0:ci0 + wa])
                        nc.sync.dma_start(XFo[64:128, 0:wb],
                                          pl_d[img, io, :, ci0 + W:ci0 + W + wb])

                        peb = [pps.tile([128, 512], f32, tag=f"pe{g}",
                                        name=f"pe{g}") for g in range(NG)]
                        pob = [pps.tile([128, 512], f32, tag=f"po{g}",
                                        name=f"po{g}") for g in range(NG)]

                        te = ("full",) if ie not in TWO_TERM else ("hi", "lo")
                        to = ("full",) if io not in TWO_TERM else ("hi", "lo")

                        # pair taps kh=0,1: stationary-major, 4+4 matmuls
                        # per LDW (redundant reloads deduped post-pass)
                        first = True
                        for kw in range(3):
                            for term in te:
                                s = wt[f"pair_{term}"][:, kw * 128:(kw + 1) * 128]
                                for g in range(NG):
                                    nc.tensor.matmul(
                                        peb[g][:, 0:grs[g] * HOUT], s,
                                        mv(XFe, 0, 128, 4 * g * W, grs[g], kw),
                                        start=first and term == te[0],
                                        stop=False)
                            for term in to:
                                s = wt[f"pair_{term}"][:, kw * 128:(kw + 1) * 128]
                                for g in range(NG):
                                    nc.tensor.matmul(
                                        pob[g][:, 0:grs[g] * HOUT], s,
                                        mv(XFo, 0, 128, 4 * g * W, grs[g], kw),
                                        start=first and term == to[0],
                                        stop=False)
                            first = False

                        # solo taps kh=2: even plane on rows 0-63 (@+224+kw),
                        # odd plane on rows 64-127 (@+112+kw); e/o interleaved
                        # so the PE co-schedules the row halves. Group-major
                        # so bank stops stagger and drains pipeline early.
                        ne, no = len(te), len(to)
                        tqe = pq.tile([128, CCMAX], f32, tag="tqe")
                        tqo = pq.tile([128, CCMAX], f32, tag="tqo")
                        s_e, s_o = SCALES[ie], SCALES[io]
                        for g in range(NG):
                            gn = grs[g] * HOUT
                            for kw in range(3):
                                for ti in range(max(ne, no)):
                                    if ti < ne:
                                        s = wt[f"solo_{te[ti]}"][0:64,
                                                                 kw * 128:(kw + 1) * 128]
                                        nc.tensor.matmul(
                                            peb[g][:, 0:gn], s,
                                            mv(XFe, 0, 64, 4 * g * W + 224,
                                               grs[g], kw),
                                            start=False,
                                            stop=(kw == 2 and ti == ne - 1))
                                    if ti < no:
                                        s = wt[f"solo_{to[ti]}"][64:128,
                                                                 kw * 128:(kw + 1) * 128]
                                        nc.tensor.matmul(
                                            pob[g][:, 0:gn], s,
                                            mv(XFo, 64, 128, 4 * g * W + W,
                                               grs[g], kw),
                                            start=False,
                                            stop=(kw == 2 and ti == no - 1))
                            # drain bank g immediately: even plane on ACT,
                            # odd plane on DVE
                            gsl = slice(g * 440, g * 440 + gn)
                            nc.scalar.activation(
                                tqe[:, gsl], peb[g][:, 0:gn],
                                mybir.ActivationFunctionType.Copy,
                                bias=MMAGIC * s_e, scale=s_e)
                            nc.vector.tensor_scalar(
                                tqo[:, gsl], pob[g][:, 0:gn],
                                s_o, MMAGIC * s_o,
                                AluOpType.mult, AluOpType.add)

                        # deferred accumulate: previous pair's ops go behind
                        # this pair's PSUM-freeing drains in the DVE FIFO
                        if pending is not None:
                            emit_accums(pending)
                            if pending_out is not None:
                                oimg, or0, occ, oacc = pending_out
                                nc.sync.dma_start(
                                    out_d[oimg, :, or0 * HOUT:or0 * HOUT + occ],
                                    oacc[:, 0:occ])
                                pending_out = None
                        pending = (acc, tqe, tqo, ie, io, ccols)
                        if pi == 3:
                            pending_out = (img, row0, ccols, acc)

            emit_accums(pending)
            oimg, or0, occ, oacc = pending_out
            nc.sync.dma_start(out_d[oimg, :, or0 * HOUT:or0 * HOUT + occ],
                              oacc[:, 0:occ])

    nd = _dedupe_ldweights(nc)
    _split_sync_waits(nc)
    if os.environ.get("V3_DEBUG"):
        print(f"[v3] deduped {nd} ldweights")
    return nc


def _prep(x, weight, bias):
    xi = np.clip(x, -128, 127).astype(np.int8)
    # bit-planes as bf16 bit patterns (1.0 = 0x3F80), flat per image
    xf = xi.reshape(16, 64, FL)
    planes = np.zeros((16, 8, 64, LW), np.float16)
    for i in range(8):
        planes[:, i, :, 0:FL] = ((xf >> i) & 1).astype(np.float16)

    w8 = (np.asarray(weight, np.float32) / np.float32(8.0)).astype(np.float32)
    wp = {k: v.astype(np.float16) for k, v in _pack_weights(w8).items()}
    c0 = (np.float32(MMAGIC * SCALES[0])
          - np.asarray(bias, np.float32)).reshape(128, 1)
    shared = {**{k: np.ascontiguousarray(v) for k, v in wp.items()},
              "c0": np.ascontiguousarray(c0.astype(np.float32))}
    in_maps = []
    for c in range(NCORES):
        m = dict(shared)
        m["pl"] = planes[c * IMGS:(c + 1) * IMGS]
        in_maps.append(m)
    return in_maps


def get_nc():
    if "nc" not in _BUILT:
        _BUILT["nc"] = _build()
    return _BUILT["nc"]


def kernel(x, weight, bias, _trace=False, _tmpdir=None):
    nc = get_nc()
    in_maps = _prep(x, weight, bias)
    br = run_bass_kernel_spmd(nc, in_maps, list(range(NCORES)),
                              trace=_trace, tmpdir=_tmpdir)
    out = np.concatenate([r["out"] for r in br.results], axis=0)
    out = out.reshape(-1, 128, HOUT, HOUT)
    if _trace:
        kernel.last_results = br
    return out.astype(np.float32)
